# revision 8
# baseline (speedup 1.0000x reference)
"""Batched attention with K/V projection on 8 TRN2 NeuronCores — v2.

Same math and host-side layouts as the baseline kernel, restructured
around the measured TensorEngine LDWEIGHTS behavior: a matmul whose
stationary operand differs from the previous matmul's pays an
unoverlapped ~107-130ns weight load, while consecutive matmuls sharing
one stationary operand run at the ~200ns/MM floor (N=512). Every
accumulation is therefore expressed as interleaved PSUM groups with the
contraction chunk OUTER and the stationary-reuse dim INNER:

  K-proj: for dc: for hc: for st(2): MM  (weight wk[dc,hc] x2 reuse)
  QK:     for kvc: for hc: for qt(4): MM (weight kT[hc,kvc] x4 reuse)
  V-proj: for dc: for kvc(8): MM          (no reuse possible: the
          stationary operand must be statesT to get kv on partitions)
  PV:     for hc: for kvc: for qt(4): MM (weight v[kvc,hc] x4 reuse)

Other deltas vs baseline:
  - QK runs for ALL q-tiles right after K-proj (V-proj deferred), so E
    tiles for the whole row block are live in SBUF (64 x [128,512] bf16).
  - The mask multiply runs in place in PSUM (no tmp SBUF roundtrip);
    Exp reads PSUM directly. One batched [128,2048] mask DMA per
    kv-chunk (the HWDGE issues dma_starts serially at ~0.63us each).
  - The softmax denominator chain (sum of E tiles) is NOT on the QK
    phase's critical path: the 60 DVE bf16 adds are interleaved with the
    V-proj bias adds in DVE FIFO order so they drain during V-proj
    matmuls without blocking any half-wave's PSUM recycle.
  - S = ones^T @ acc (4 tiny MMs) right after V-proj; reciprocal +
    partition-broadcast overlap PV's first chain.
  - Startup: constants DMA'd first, activation tables preloaded with
    dummy ops, and 48 dependency-free warm-up matmuls keep the PE's HAM
    clock gate at 8/8 through the initial DMA wait.
"""

import os
import numpy as np
import ml_dtypes

B, SQ, SKV, DIN, H = 8, 2048, 2048, 1024, 512
P = 128
HC = H // P      # 4  h-chunks of 128
DC = DIN // P    # 8  d-chunks of 128
KVC = SKV // P   # 16 kv-chunks of 128
QT = SQ // 512   # 4  q-tiles of 512
ST = SKV // 512  # 4  kv-tiles of 512

LAST_EXEC_NS = None
LAST_RESULTS = None
_NC = None


def _build(repeat=1):
    import contextlib
    import concourse.bacc as bacc
    import concourse.tile as tile
    import concourse.mybir as mybir

    f32 = mybir.dt.float32
    bf16 = mybir.dt.bfloat16
    Exp = mybir.ActivationFunctionType.Exp
    Ident = mybir.ActivationFunctionType.Identity

    nc = bacc.Bacc("TRN2", target_bir_lowering=False, debug=False,
                   num_devices=8, num_swdge_queues=4)
    qT_d = nc.dram_tensor("qT", [H, SQ], bf16, kind="ExternalInput").ap()
    sT_d = nc.dram_tensor("sT", [DIN, SKV], bf16, kind="ExternalInput").ap()
    mT_d = nc.dram_tensor("mT", [SKV, SQ], bf16, kind="ExternalInput").ap()
    wk_d = nc.dram_tensor("wk", [DIN, H], bf16, kind="ExternalInput").ap()
    wv_d = nc.dram_tensor("wv", [DIN, H], bf16, kind="ExternalInput").ap()
    bk_d = nc.dram_tensor("bk", [H], f32, kind="ExternalInput").ap()
    bv_d = nc.dram_tensor("bv", [H], f32, kind="ExternalInput").ap()
    out_d = nc.dram_tensor("out", [H, SQ], bf16, kind="ExternalOutput").ap()

    with tile.TileContext(nc) as tc:
        with tc.tile_pool(name="const", bufs=1) as cpool, \
             tc.tile_pool(name="big", bufs=1) as big, \
             tc.tile_pool(name="masks", bufs=2) as mpool, \
             tc.tile_pool(name="vtb", bufs=4) as vtbpool, \
             tc.tile_pool(name="epool", bufs=64) as epool, \
             tc.tile_pool(name="osb", bufs=2) as opool, \
             tc.tile_pool(name="ivb", bufs=4) as ipool, \
             tc.tile_pool(name="ps", bufs=8, space="PSUM") as psp, \
             (tc.For_i(0, repeat, 1, hint_engines=(
                  mybir.EngineType.PE, mybir.EngineType.DVE,
                  mybir.EngineType.Activation, mybir.EngineType.Pool,
                  mybir.EngineType.SP))
              if repeat > 1 else contextlib.nullcontext()):

            # ---- resident inputs. Order = DMA queue order: the K-proj
            # wave 0 stream (wk[dc] + statesT[dc, 0:1024]) goes first so
            # the PE starts as soon as the first pieces land.
            wk_sb = big.tile([P, DC, H], bf16)
            wv_sb = big.tile([P, DC, H], bf16)
            st_sb = big.tile([P, DC, SKV], bf16)
            qT_sb = big.tile([P, HC, SQ], bf16)

            # constants first: they are tiny, but the HWDGE issues DMAs at
            # ~0.63us each in order, and the K-proj ScalarE bias needs bk.
            ones = cpool.tile([P, 1], bf16)
            nc.any.memset(ones, 1.0)
            # preload both activation-function tables during the DMA ramp
            actwarm = cpool.tile([1, 1], f32)
            nc.scalar.activation(actwarm, ones[0:1, 0:1], Ident)
            nc.scalar.activation(actwarm, ones[0:1, 0:1], Exp)

            # HAM warm-up: dependency-free matmuls on constant tiles keep
            # the PE busy during the initial DMA wait, so the clock gate is
            # at 8/8 when the first projection matmul issues (the activity
            # window needs ~3.4us of sustained work).
            wuw = cpool.tile([P, P], bf16)
            nc.any.memset(wuw, 0.0)
            wur = cpool.tile([P, 64], bf16)
            nc.any.memset(wur, 0.0)
            for i in range(48):
                wps = psp.tile([P, 64], f32, tag="ps", name=f"wu{i}")
                nc.tensor.matmul(wps, wuw, wur, start=True, stop=True)

            nc.sync.dma_start(wk_sb[:, 0, 0:256], wk_d[0:P, 0:256])
            nc.sync.dma_start(st_sb[:, 0, 0:512], sT_d[0:P, 0:512])
            nc.sync.dma_start(wk_sb[:, 0, 256:512], wk_d[0:P, 256:512])
            nc.sync.dma_start(st_sb[:, 0, 512:1024], sT_d[0:P, 512:1024])
            bk_sb = cpool.tile([P, HC], f32)
            bv_sb = cpool.tile([P, HC], f32)
            for dc in range(1, DC):
                nc.sync.dma_start(wk_sb[:, dc], wk_d[dc * P:(dc + 1) * P])
                nc.sync.dma_start(st_sb[:, dc, 0:1024],
                                  sT_d[dc * P:(dc + 1) * P, 0:1024])
                if dc == 4:
                    # constants ride behind the dc<=4 stream: the HWDGE
                    # issues serially at ~0.63us each, and bk is first
                    # needed by the kT bias ACT at ~12us.
                    nc.sync.dma_start(bk_sb,
                                      bk_d.rearrange("(c p) -> p c", p=P))
                    nc.sync.dma_start(bv_sb,
                                      bv_d.rearrange("(c p) -> p c", p=P))
            for dc in range(DC):
                nc.sync.dma_start(st_sb[:, dc, 1024:2048],
                                  sT_d[dc * P:(dc + 1) * P, 1024:2048])
            for hc in range(HC):
                nc.sync.dma_start(qT_sb[:, hc], qT_d[hc * P:(hc + 1) * P])
            for dc in range(DC):
                nc.sync.dma_start(wv_sb[:, dc], wv_d[dc * P:(dc + 1) * P])

            kT_sb = big.tile([P, HC, SKV], bf16)
            v_sb = big.tile([P, KVC, H], bf16)

            # ---- K-projection: kv-half outer (matches the statesT DMA
            # stream rate), then half-waves of (2 hc x 2 st-tiles) = 4
            # chains, weight wk[dc,hc] shared by 2 consecutive MMs. 4 PSUM
            # slots per half-wave so the ScalarE bias/cast drain of
            # half-wave i overlaps half-wave i+1's matmuls.
            for w in range(2):
                for hp in range(2):
                    kps = [psp.tile([P, 512], f32, tag="ps",
                                    name=f"kps{w}{hp}_{i}") for i in range(4)]
                    for dc in range(DC):
                        for hi in range(2):
                            hc = 2 * hp + hi
                            wt = wk_sb[:, dc, hc * P:(hc + 1) * P]
                            for sti in range(2):
                                col = (2 * w + sti) * 512
                                nc.tensor.matmul(kps[hi * 2 + sti], wt,
                                                 st_sb[:, dc, col:col + 512],
                                                 start=(dc == 0),
                                                 stop=(dc == DC - 1))
                    for hi in range(2):
                        hc = 2 * hp + hi
                        for sti in range(2):
                            col = (2 * w + sti) * 512
                            nc.scalar.activation(
                                kT_sb[:, hc, col:col + 512],
                                kps[hi * 2 + sti], Ident,
                                bias=bk_sb[:, hc:hc + 1])

            # ---- QK for ALL q-tiles, kv-chunk by kv-chunk.
            # Weight kT[hc, kvc] shared by the 4 q-tile MMs.
            e_tiles = [[None] * KVC for _ in range(QT)]
            for kvc in range(KVC):
                sps = [psp.tile([P, 512], f32, tag="ps", name=f"sps{qt}")
                       for qt in range(QT)]
                for hc in range(HC):
                    wt = kT_sb[:, hc, kvc * P:(kvc + 1) * P]
                    for qt in range(QT):
                        nc.tensor.matmul(sps[qt], wt,
                                         qT_sb[:, hc, qt * 512:(qt + 1) * 512],
                                         start=(hc == 0), stop=(hc == HC - 1))
                mk = mpool.tile([P, SQ], bf16, tag="mask")
                nc.sync.dma_start(mk, mT_d[kvc * P:(kvc + 1) * P])
                for qt in range(QT):
                    # mask-multiply in place in PSUM (no tmp roundtrip);
                    # Exp then reads PSUM, which is faster ScalarE access
                    nc.vector.tensor_mul(sps[qt], sps[qt],
                                         mk[:, qt * 512:(qt + 1) * 512])
                    et = epool.tile([P, 512], bf16, tag="e",
                                    name=f"e{qt}_{kvc}")
                    nc.scalar.activation(et, sps[qt], Exp)
                    e_tiles[qt][kvc] = et

            # ---- deferred softmax-denominator accumulation (DVE, bf16).
            # bf16 partials: ~0.2% on S, well inside budget. Interleaved
            # with the V-proj bias adds below so the DVE FIFO never blocks
            # a half-wave's PSUM recycle: [28 acc][bias w0][16 acc]
            # [bias w1][16 acc][bias w2][bias w3].
            acc = cpool.tile([P, QT, 512], bf16)

            def emit_acc(levels):
                for kvc in levels:
                    for qt in range(QT):
                        if kvc == 1:
                            nc.vector.tensor_add(acc[:, qt], e_tiles[qt][0],
                                                 e_tiles[qt][1])
                        else:
                            nc.vector.tensor_add(acc[:, qt], acc[:, qt],
                                                 e_tiles[qt][kvc])

            emit_acc(range(1, 8))

            # ---- V-projection, transposed: produce vT [h-part, kv] with
            # wv stationary (x4 weight reuse over kv tiles, like K-proj),
            # bias on ScalarE (bv IS per-partition in this layout), then
            # DMA-transpose each piece into v_sb [kv-part, h]. The xbar
            # transpose maps row r -> (partition r%128, chunk r//128),
            # HW-verified; ~0.9us/piece rides the idle DMA queue.
            acc_batches = [range(8, 12), range(12, KVC), (), ()]
            for hc in range(HC):
                vtps = [psp.tile([P, 512], f32, tag="ps", name=f"vt{hc}_{i}")
                        for i in range(4)]
                for dc in range(DC):
                    wt = wv_sb[:, dc, hc * P:(hc + 1) * P]
                    for sti in range(ST):
                        nc.tensor.matmul(vtps[sti], wt,
                                         st_sb[:, dc,
                                               sti * 512:(sti + 1) * 512],
                                         start=(dc == 0),
                                         stop=(dc == DC - 1))
                for sti in range(ST):
                    vt = vtbpool.tile([P, 512], bf16, tag="vt",
                                      name=f"vtb{hc}_{sti}")
                    nc.scalar.activation(vt, vtps[sti], Ident,
                                         bias=bv_sb[:, hc:hc + 1])
                    nc.sync.dma_start_transpose(
                        v_sb[:, sti * 4:(sti + 1) * 4, hc * P:(hc + 1) * P],
                        vt)
                emit_acc(acc_batches[hc])

            # ---- denominators: S = ones^T @ acc per q-tile, then
            # reciprocal + broadcast overlap PV's first chain.
            inv_rows = []
            for qt in range(QT):
                S_ps = psp.tile([P, 512], f32, tag="ps", name=f"S{qt}")
                nc.tensor.matmul(S_ps[0:1, :], ones, acc[:, qt],
                                 start=True, stop=True)
                ir = opool.tile([1, 512], f32, tag="o", name=f"ir{qt}")
                nc.vector.reciprocal(ir, S_ps[0:1, :])
                inv_rows.append(ir)
            invbs = []
            for qt in range(QT):
                ib = ipool.tile([P, 512], f32, tag="invb", name=f"ib{qt}")
                nc.gpsimd.partition_broadcast(ib, inv_rows[qt])
                invbs.append(ib)

            # ---- PV: hc outer, kv-chunk weight shared by the 4 q-tiles.
            for hc in range(HC):
                ops = [psp.tile([P, 512], f32, tag="ps", name=f"op{qt}")
                       for qt in range(QT)]
                for kvc in range(KVC):
                    wt = v_sb[:, kvc, hc * P:(hc + 1) * P]
                    for qt in range(QT):
                        nc.tensor.matmul(ops[qt], wt, e_tiles[qt][kvc],
                                         start=(kvc == 0),
                                         stop=(kvc == KVC - 1))
                ot = opool.tile([P, SQ], bf16, tag="o")
                if hc < HC - 1:
                    for qt in range(QT):
                        nc.vector.tensor_mul(ot[:, qt * 512:(qt + 1) * 512],
                                             ops[qt], invbs[qt])
                    nc.sync.dma_start(out_d[hc * P:(hc + 1) * P], ot)
                else:
                    # last group: per-q-tile stores so the epilogue
                    # pipelines normalize against DMA instead of waiting
                    # for all four normalizes before one big store. The
                    # stores alternate between the SP and (idle) ScalarE
                    # DGE queues so they issue in parallel rather than
                    # serializing at ~0.63us each on one HWDGE queue.
                    for qt in range(QT):
                        nc.vector.tensor_mul(ot[:, qt * 512:(qt + 1) * 512],
                                             ops[qt], invbs[qt])
                        eng = nc.sync if qt % 2 == 0 else nc.scalar
                        eng.dma_start(
                            out_d[hc * P:(hc + 1) * P,
                                  qt * 512:(qt + 1) * 512],
                            ot[:, qt * 512:(qt + 1) * 512])

    nc.compile()
    return nc


def kernel(query, states, mask, Wk, bk, Wv, bv):
    global LAST_EXEC_NS, LAST_RESULTS, _NC
    from concourse.bass_utils import run_bass_kernel_spmd

    if _NC is None:
        _NC = _build()

    query = np.asarray(query)
    states = np.asarray(states)
    mask = np.asarray(mask)
    Wk, bk, Wv, bv = (np.asarray(x) for x in (Wk, bk, Wv, bv))
    bf = ml_dtypes.bfloat16
    scale = 1.0 / np.sqrt(np.float32(H))
    wk_b = np.ascontiguousarray(Wk.astype(bf))
    wv_b = np.ascontiguousarray(Wv.astype(bf))
    bk_f = np.ascontiguousarray(bk.astype(np.float32))
    bv_f = np.ascontiguousarray(bv.astype(np.float32))
    in_maps = []
    for b in range(B):
        in_maps.append({
            "qT": np.ascontiguousarray((query[b].T * scale).astype(bf)),
            "sT": np.ascontiguousarray(states[b].T.astype(bf)),
            "mT": np.ascontiguousarray(mask[b].T.astype(bf)),
            "wk": wk_b, "wv": wv_b, "bk": bk_f, "bv": bv_f,
        })

    trace = os.environ.get("BASS_KERNEL_TRACE", "0") not in ("", "0", "false")
    try:
        res = run_bass_kernel_spmd(_NC, in_maps, core_ids=list(range(B)), trace=trace)
    except ModuleNotFoundError:
        os.environ["BASS_NEVER_TRACE"] = "1"
        res = run_bass_kernel_spmd(_NC, in_maps, core_ids=list(range(B)))
    LAST_EXEC_NS = res.exec_time_ns
    LAST_RESULTS = res
    out = np.stack([res.results[b]["out"].T for b in range(B)])
    return np.ascontiguousarray(out.astype(np.float32))
